# revision 26
# baseline (speedup 1.0000x reference)
"""CRF-RNN layer (nn_CrfRnnLayer) as a Trainium2 Bass kernel on 8 NeuronCores.

Math
----
The reference iterates, for q in R^{2xN} (N=3136 pixels, 2 classes):
    s         = softmax(q, axis=0)            (or s = unaries on iter 0)
    sp_out    = (s @ K_sp) / (K_sp @ 1)
    bl_out    = (s @ K_bl) / (K_bl @ 1)
    message   = sp_w @ sp_out + bl_w @ bl_out
    q         = unaries - compat @ message
Both rows of s sum to one (softmax; unaries too), and both kernel matrices
are symmetric, so the whole update collapses to a scalar recursion on
d = q[0] - q[1]:
    s0   = sigmoid(d)                        (s = [s0, 1-s0])
    v[i] = sum_j C[j,i] * s0[j]
    d    = U - v
with C = A*K_sp/nsp + B*K_bl/nbl (column-normalized), U = (1-2u) - G, and
A, B, G scalars derived from the 2x2 weight matrices.  The final output is
softmax(q)[1] = sigmoid(-d).

Device strategy (8 cores)
-------------------------
Column-shard C: core c owns columns i in [392c, 392(c+1)).  Each core:
  1. builds its [3136 x 392] block of exp(-0.5*sqdist) for both kernels
     directly on-chip: the full exponent comes out of one PE matmul over
     augmented features (D=4 spatial / D=5 bilateral), then ScalarE Exp;
  2. column sums via ones-matmuls -> reciprocal -> merge the two kernels
     into one C block (3 DVE ops with a PE-broadcast row);
  3. iterates: 28 accumulating matvec matmuls + row ops + sigmoid, then a
     1.5KB AllGather redistributes s0 to all cores (j is tiled as
     j = p*28 + t so the gathered vector DMA-rearranges into the [112,28]
     lhsT layout with per-partition contiguous reads).
Only the last iteration skips the collective and writes sigmoid(-d).
"""

import sys

for _p in ("/root/.axon_site/_ro/trn_rl_repo", "/opt/trn_rl_repo"):
    if _p not in sys.path:
        sys.path.append(_p)

import numpy as np

import concourse.bass as bass  # noqa: F401  (registers AP types)
import concourse.tile as tile
from concourse import bacc, mybir, bass_utils

F32 = mybir.dt.float32
BF16 = mybir.dt.bfloat16
AF = mybir.ActivationFunctionType
ALU = mybir.AluOpType

H = W = 56
N = H * W            # 3136 pixels
NC = 8               # cores
SHARD = N // NC      # 392 columns per core
P = 112              # j partition-tile height (112*28 == 3136)
T = 28               # number of j tiles
ITERS = 10
TH_ALPHA, TH_BETA, TH_GAMMA = 160.0, 3.0, 3.0

_BANK = 512          # one PSUM bank, in f32 elements
_GRP = 3             # exponent tiles batched per ScalarE Exp call
_WARM = 64           # HAM warm-keeping matmuls issued under each collective
_CSC = 4096.0        # fp8 prescale on C (exact power of two)
_CPAD = 392          # fp8 cmat per-tile column pitch
_GH = T // 2         # DoubleRow groups per matvec (14)


def _build(a_val: float, b_val: float) -> "bacc.Bacc":
    nc = bacc.Bacc("TRN2", target_bir_lowering=False, debug=False,
                   num_devices=NC)

    F8 = mybir.dt.float8e4
    fa_in = nc.dram_tensor("fa", [128, 2 * N], BF16, kind="ExternalInput").ap()
    fb_in = nc.dram_tensor("fb", [128, 2 * SHARD], BF16,
                           kind="ExternalInput").ap()
    u_in = nc.dram_tensor("u", [2, SHARD], BF16, kind="ExternalInput").ap()
    s0_in = nc.dram_tensor("s0", [P, 32], F8, kind="ExternalInput").ap()
    onec_in = nc.dram_tensor("onec", [P, 1], BF16, kind="ExternalInput").ap()
    one2_in = nc.dram_tensor("one2", [2, 1], BF16, kind="ExternalInput").ap()
    oner_in = nc.dram_tensor("oner", [1, P], F32, kind="ExternalInput").ap()
    out = nc.dram_tensor("out", [1, SHARD], F32, kind="ExternalOutput").ap()
    sink = nc.dram_tensor("sink", [1, 1], F32, kind="ExternalOutput").ap()
    # collective buffers: Local input, Shared output (fast HBM-HBM path)
    di_d = [nc.dram_tensor(f"di{k}", [SHARD], F8, kind="Internal").ap()
            for k in range(2)]
    do_d = [nc.dram_tensor(f"do{k}", [N], F8, kind="Internal",
                           addr_space="Shared").ap()
            for k in range(2)]

    groups = [list(range(g, min(g + _GRP, T))) for g in range(0, T, _GRP)]

    with tile.TileContext(nc) as tc:
        with (
            tc.tile_pool(name="const", bufs=1) as cpool,
            tc.tile_pool(name="emat", bufs=1) as epool,
            tc.tile_pool(name="row", bufs=2) as rpool,
            tc.tile_pool(name="sten", bufs=2) as spool,
            tc.tile_pool(name="dram", bufs=2, space="DRAM") as dpool,
        ):
            # exponent-feature operands are zero-padded to 128 contraction
            # rows: a 4/5-row matmul doesn't register as PE activity, so the
            # HAM clock gate keeps the whole construction at 1.2 GHz.
            # Host sends the pad rows pre-zeroed (DMA engines are idle at
            # startup, the DVE memsets were on the critical path), and the
            # four operands ride in two DMAs: [bilateral | spatial].
            fa_t = cpool.tile([128, 2 * N], BF16, tag="fa")
            nc.sync.dma_start(fa_t[:], fa_in[:])
            fb_t = cpool.tile([128, 2 * SHARD], BF16, tag="fb")
            nc.sync.dma_start(fb_t[:], fb_in[:])
            u_t = cpool.tile([2, SHARD], BF16, tag="u")
            nc.sync.dma_start(u_t[:], u_in[:])
            s0_t = cpool.tile([P, 32], F8, tag="s0")
            nc.sync.dma_start(s0_t[:], s0_in[:])
            ones_col = cpool.tile([P, 1], BF16, tag="onec")
            nc.sync.dma_start(ones_col[:], onec_in[:])
            ones2 = cpool.tile([2, 1], BF16, tag="one2")
            nc.sync.dma_start(ones2[:], one2_in[:])
            ones_row = cpool.tile([1, P], F32, tag="oner")
            nc.sync.dma_start(ones_row[:], oner_in[:])

            # throwaway AllGather: absorbs the ~40us collectives entry
            # barrier + ~18us first-op warmup under the construction phase
            nc.sync.dma_start(
                di_d[0][:], s0_in[:].rearrange("p t -> (p t)")[0:SHARD])
            nc.gpsimd.collective_compute(
                "AllGather", ALU.bypass,
                replica_groups=[list(range(NC))],
                ins=[di_d[0][:].opt()], outs=[do_d[0][:].opt()],
            )

            ebl = epool.tile([P, T * SHARD], BF16, tag="ebl")
            cmat = epool.tile([P, T * _CPAD], F8, tag="cmat")

            # ---- phase 1: exponent matmuls + exp + column sums ----
            # bilateral pass first: its serial tail (colsum -> recip ->
            # rb broadcast -> ebl*rb) overlaps the spatial pass, whose exp
            # (with ln(CSC*|A|/nsp) absorbed as two feature rows) lands
            # directly in the fp8 cmat.
            with (
                tc.tile_pool(name="psg", bufs=2, space="PSUM") as psg,
                tc.tile_pool(name="pss", bufs=1, space="PSUM") as pss,
            ):
                cs_bl = pss.tile([1, SHARD], F32, tag="cs_bl")

                def exp_pass(aoff, boff, dst3, cs, scale):
                    for grp in groups:
                        pg = psg.tile([P, _GRP * _BANK], F32, tag="grp")
                        for k, t in enumerate(grp):
                            nc.tensor.matmul(
                                pg[:, k * _BANK : k * _BANK + SHARD],
                                fa_t[:, aoff + t * P : aoff + (t + 1) * P],
                                fb_t[:, boff : boff + SHARD],
                                start=True, stop=True,
                                skip_group_check=True,
                            )
                        ln = len(grp)
                        src = pg[:].rearrange("p (k f) -> p k f", f=_BANK)[
                            :, 0:ln, 0:SHARD]
                        nc.scalar.activation(dst3[:, grp[0] : grp[0] + ln, :],
                                             src, AF.Exp, scale=scale)
                        for t in (grp if cs is not None else []):
                            nc.tensor.matmul(
                                cs[:],
                                ones_col[:],
                                ebl[:, t * SHARD : (t + 1) * SHARD],
                                start=(t == 0), stop=(t == T - 1),
                                skip_group_check=True,
                            )

                b3 = ebl[:].rearrange("p (k f) -> p k f", f=SHARD)
                c3 = cmat[:].rearrange("p (k f) -> p k f", f=_CPAD
                                       )[:, :, 0:SHARD]
                exp_pass(0, 0, b3, cs_bl, -1.0 / 6.0)

                rb_row = cpool.tile([1, SHARD], F32, tag="rb")
                nc.vector.reciprocal(rb_row[:], cs_bl[:])
                nc.scalar.mul(rb_row[:], rb_row[:], float(b_val * _CSC))

                rb_bc = pss.tile([P, SHARD], F32, tag="rbbc")
                nc.tensor.matmul(rb_bc[:], ones_row[:], rb_row[:],
                                 start=True, stop=True, skip_group_check=True)
                rb_sb = cpool.tile([P, SHARD], BF16, tag="rbsb")
                nc.vector.tensor_copy(rb_sb[:], rb_bc[:])

                exp_pass(N, SHARD, c3, None, 1.0 / 9.0)

                # pre-load the sigmoid ACT table behind the merge: the
                # table RAM holds one set, so loading it before the Exp
                # passes would just thrash
                pre_sg = cpool.tile([1, 1], F32, tag="presg")
                nc.scalar.activation(pre_sg[:], ones_row[0:1, 0:1],
                                     AF.Sigmoid)

                # merge C = Esp' +/- Ebl*RB, split in pair-halves so the
                # first matvec groups can start before the second half lands
                c4 = cmat[:].rearrange("p (e q f) -> p e q f", q=_GH,
                                       f=_CPAD)[:, :, :, 0:SHARD]
                b4 = ebl[:].rearrange("p (e q f) -> p e q f", q=_GH, f=SHARD)
                r4 = rb_sb[:].rearrange("p (a b f) -> p a b f", a=1, b=1)
                for h in range(2):
                    q0, q1 = (h * _GH) // 2, ((h + 1) * _GH) // 2
                    nq = q1 - q0
                    bh = b4[:, :, q0:q1, :]
                    ch = c4[:, :, q0:q1, :]
                    nc.vector.tensor_mul(
                        bh, bh, r4.broadcast_to([P, 2, nq, SHARD]))
                    if a_val >= 0.0:
                        nc.vector.tensor_add(ch, ch, bh)
                    else:
                        nc.vector.tensor_sub(ch, bh, ch)

            # ---- phase 3: CRF mean-field iterations ----
            # psum accumulates -CSC*d = (-CSC*u seed) + sum_t CSC*C^T s.
            # fp8 DoubleRow matvec: group g contracts j-tiles (g, g+_GH);
            # s is stored [P, 2, 16] (14 used + 2 pad, 16B subtile step).
            with (
                tc.tile_pool(name="psv", bufs=2, space="PSUM") as psv,
                tc.tile_pool(name="psd", bufs=1, space="PSUM") as psd,
            ):
                dummy = psd.tile([1, SHARD], F32, tag="dummy")
                cm3 = cmat[:].rearrange("p (e g) -> p e g", e=2,
                                        g=_GH * _CPAD)
                DR = mybir.MatmulPerfMode.DoubleRow

                def seed(vt):
                    nc.tensor.matmul(
                        vt[:], ones2[:], u_t[:],
                        start=True, stop=False, skip_group_check=True,
                    )

                s_cur = s0_t
                v = psv.tile([1, SHARD], F32, tag="v")
                seed(v)
                for it in range(ITERS):
                    s3 = s_cur[:].rearrange("p (e g) -> p e g", e=2, g=16)
                    for g in range(_GH):
                        nc.tensor.matmul(
                            v[:],
                            s3[:, :, g : g + 1],
                            cm3[:, :, g * _CPAD : g * _CPAD + SHARD],
                            start=False, stop=(g == _GH - 1),
                            perf_mode=DR,
                            skip_group_check=True,
                        )
                    if it < ITERS - 1:
                        s_row = rpool.tile([1, SHARD], F8, tag="srow")
                        nc.scalar.activation(s_row[:], v[:], AF.Sigmoid,
                                             scale=-1.0 / _CSC)
                        di, do = di_d[it % 2], do_d[it % 2]
                        # scalar-issued DMA: no cross-engine semaphore hop
                        # between the sigmoid and the collective input
                        nc.scalar.dma_start(
                            di[:].rearrange("(a b) -> a b", a=1), s_row[:])
                        nc.gpsimd.collective_compute(
                            "AllGather", ALU.bypass,
                            replica_groups=[list(range(NC))],
                            ins=[di[:].opt()], outs=[do[:].opt()],
                        )
                        # keep the PE HAM-warm through the collective gap.
                        # The first ("linker") matmul reads s_row, so the
                        # whole WAW-chained dummy block is ordered after the
                        # sigmoid — it cannot interleave into the matvec
                        # accumulation and delay v's ready semaphore.  The
                        # next iteration's u-seed hides under the collective
                        # too, right behind the linker.
                        nc.tensor.matmul(
                            dummy[:], s_row[0:1, 0:1], s_row[:],
                            start=True, stop=True, skip_group_check=True,
                        )
                        v = psv.tile([1, SHARD], F32, tag="v")
                        seed(v)
                        for w in range(_WARM):
                            c0 = (w % T) * _CPAD
                            nc.tensor.matmul(
                                dummy[:],
                                s_cur[:, (w % 28) : (w % 28) + 1],
                                cmat[:, c0 : c0 + SHARD],
                                start=True, stop=True,
                                skip_group_check=True,
                            )
                        s_nxt = spool.tile([P, 32], F8, tag="s")
                        nc.sync.dma_start(
                            s_nxt[:].rearrange("p (e g) -> p e g",
                                               e=2, g=16)[:, :, 0:_GH],
                            do[:].rearrange("(p e g) -> p e g", e=2, g=_GH))
                        s_cur = s_nxt
                    else:
                        o_row = rpool.tile([1, SHARD], F32, tag="orow")
                        nc.scalar.activation(o_row[:], v[:], AF.Sigmoid,
                                             scale=1.0 / _CSC)
                        nc.scalar.dma_start(out[:], o_row[:])
                sink_row = rpool.tile([1, 1], F32, tag="sink")
                nc.vector.tensor_copy(sink_row[:], dummy[0:1, 0:1])
                nc.sync.dma_start(sink[:], sink_row[:])

    nc.compile()
    return nc


def _host_prep(inputs, spatial_ker_weights, bilateral_ker_weights,
               compatibility_matrix):
    unary = np.asarray(inputs[0], dtype=np.float64)
    gray = np.asarray(inputs[1], dtype=np.float64)
    sp_w = np.asarray(spatial_ker_weights, dtype=np.float64)
    bl_w = np.asarray(bilateral_ker_weights, dtype=np.float64)
    compat = np.asarray(compatibility_matrix, dtype=np.float64)

    dsp = sp_w[:, 0] - sp_w[:, 1]
    dbl = bl_w[:, 0] - bl_w[:, 1]
    c0 = sp_w[:, 1] + bl_w[:, 1]
    dc = compat[0, :] - compat[1, :]
    a_val = float(dc @ dsp)
    b_val = float(dc @ dbl)
    g_val = float(dc @ c0)

    ys, xs = np.meshgrid(np.arange(H, dtype=np.float64),
                         np.arange(W, dtype=np.float64), indexing="ij")
    x = xs.ravel()
    y = ys.ravel()
    gf = gray.ravel() * 255.0

    import ml_dtypes
    _mld = ml_dtypes
    one = np.ones(N, dtype=np.float64)

    def bf(v):
        return np.asarray(v, dtype=_mld.bfloat16).astype(np.float64)

    def split3(v):
        a = bf(v)
        b = bf(v - a)
        c = bf(v - a - b)
        return a, b, c

    # spatial norm is a Kronecker product: nsp[(y,x)] = ry[y]*rx[x]
    idx = np.arange(H, dtype=np.float64)
    g1d = np.exp(-0.5 * ((idx[None, :] - idx[:, None]) / TH_GAMMA) ** 2)
    r1d = g1d.sum(axis=1)
    nsp = (r1d[y.astype(int)] * r1d[x.astype(int)])

    # spatial exponent in bf16-exact integer arithmetic, scaled by 1/9 at
    # the Exp activation; the fp8 prescale and the column norm are folded
    # in as a 9*ln(CSC*|A|/nsp) hi/lo feature pair:
    # presc = xj*xi + yj*yi - (x^2+y^2)/2 terms + ln rows
    ssp_i = 0.5 * (x * x + y * y)                 # multiples of 0.5
    sp_hi = bf(-ssp_i)
    sp_lo = -ssp_i - sp_hi                        # exact in bf16
    lr = 9.0 * np.log(np.maximum(4096.0 * abs(a_val) / nsp, 1e-280))
    lr = np.maximum(lr, -2000.0)
    lr_hi = bf(lr)
    lr_lo = lr - lr_hi

    asp_g = np.stack([x, y, one, one, sp_hi, sp_lo, one, one], axis=0)
    bsp_g = np.stack([x, y, sp_hi, sp_lo, one, one, lr_hi, lr_lo], axis=0)

    # bilateral exponent on bf16 features: presc = ssq_i + ssq_j
    # - 2*(xp_i xp_j + yp_i yp_j + g_i g_j), scaled by -1/6 at the Exp.
    # g and ssq are 3-way bf16 splits so every product is exact in f32;
    # validated max exponent error ~2e-3.
    s3f = np.sqrt(3.0)
    xp = bf(s3f * x / TH_ALPHA)
    yp = bf(s3f * y / TH_ALPHA)
    g1, g2, g3 = split3(gf)
    gs = g1 + g2 + g3
    ssq = xp * xp + yp * yp + gs * gs
    s1, s2, sr = split3(ssq)

    abl_g = np.stack([xp, yp, g1, g1, g2, g1, g3, g2,
                      s1, s2, sr, one, one, one], axis=0)
    bbl_g = np.stack([-2 * xp, -2 * yp, -2 * g1, -2 * g2, -2 * g1,
                      -2 * g3, -2 * g1, -2 * g2,
                      one, one, one, s1, s2, sr], axis=0)

    # device j-tiling: lhsT column t*P + p  <->  global j = p*T + t
    j_order = np.arange(N).reshape(P, T).T.ravel()

    def pad128(m, cols):
        out = np.zeros((128, cols), dtype=_mld.bfloat16)
        out[: m.shape[0]] = m.astype(_mld.bfloat16)
        return out

    # fa/fb pack [bilateral | spatial] so each rides in a single DMA
    fa_d = np.concatenate(
        [pad128(abl_g[:, j_order], N), pad128(asp_g[:, j_order], N)], axis=1)

    def hilo(v):
        hi = np.asarray(v, dtype=_mld.bfloat16).astype(np.float64)
        lo = np.asarray(v - hi, dtype=_mld.bfloat16)
        return np.stack([hi.astype(_mld.bfloat16), lo], axis=0)

    u_flat = unary.ravel()
    u_full = hilo(-4096.0 * ((1.0 - 2.0 * u_flat) - g_val))    # [2, N] bf16
    s0_pt = (1.0 - u_flat).reshape(P, T)
    s0_dev = np.zeros((P, 32), dtype=ml_dtypes.float8_e4m3fn)
    s0_dev[:, 0:14] = s0_pt[:, 0:14].astype(ml_dtypes.float8_e4m3fn)
    s0_dev[:, 16:30] = s0_pt[:, 14:28].astype(ml_dtypes.float8_e4m3fn)

    in_maps = []
    for c in range(NC):
        sl = slice(c * SHARD, (c + 1) * SHARD)
        in_maps.append({
            "fa": fa_d,
            "fb": np.concatenate([pad128(bbl_g[:, sl], SHARD),
                                  pad128(bsp_g[:, sl], SHARD)], axis=1),
            "u": np.ascontiguousarray(u_full[:, sl]),
            "s0": s0_dev,
            "onec": np.ones((P, 1), dtype=ml_dtypes.bfloat16),
            "one2": np.ones((2, 1), dtype=ml_dtypes.bfloat16),
            "oner": np.ones((1, P), dtype=np.float32),
        })
    return a_val, b_val, in_maps


_CACHE = {}


def kernel(inputs, spatial_ker_weights, bilateral_ker_weights,
           compatibility_matrix, _want_results=False):
    a_val, b_val, in_maps = _host_prep(
        inputs, spatial_ker_weights, bilateral_ker_weights,
        compatibility_matrix)

    key = (a_val, b_val)
    if key not in _CACHE:
        _CACHE[key] = _build(a_val, b_val)
    nc = _CACHE[key]

    res = bass_utils.run_bass_kernel_spmd(nc, in_maps, list(range(NC)))
    prob = np.concatenate([res.results[c]["out"][0] for c in range(NC)])
    out = prob.reshape(1, H, W).astype(np.float32)
    if _want_results:
        return out, nc, in_maps
    return out


if __name__ == "__main__":
    rng = np.random.default_rng(0)
    demo = {
        "inputs": rng.random((2, H, W)).astype(np.float32),
        "spatial_ker_weights":
            (rng.random((2, 2)).astype(np.float32) - 0.5) * 0.1,
        "bilateral_ker_weights":
            (rng.random((2, 2)).astype(np.float32) - 0.5) * 0.1,
        "compatibility_matrix":
            (rng.random((2, 2)).astype(np.float32) - 0.5) * 0.1,
    }
    print(kernel(**demo).shape)



# revision 38
# speedup vs baseline: 1.0854x; 1.0854x over previous
"""CRF-RNN layer (nn_CrfRnnLayer) as a Trainium2 Bass kernel on 8 NeuronCores.

Math
----
The reference iterates, for q in R^{2xN} (N=3136 pixels, 2 classes):
    s         = softmax(q, axis=0)            (or s = unaries on iter 0)
    sp_out    = (s @ K_sp) / (K_sp @ 1)
    bl_out    = (s @ K_bl) / (K_bl @ 1)
    message   = sp_w @ sp_out + bl_w @ bl_out
    q         = unaries - compat @ message
Both rows of s sum to one (softmax; unaries too), and both kernel matrices
are symmetric, so the whole update collapses to a scalar recursion on
d = q[0] - q[1]:
    s0   = sigmoid(d)                        (s = [s0, 1-s0])
    v[i] = sum_j C[j,i] * s0[j]
    d    = U - v
with C = A*K_sp/nsp + B*K_bl/nbl (column-normalized), U = (1-2u) - G, and
A, B, G scalars derived from the 2x2 weight matrices.  The final output is
softmax(q)[1] = sigmoid(-d).

Device strategy (8 cores)
-------------------------
Column-shard C: core c owns columns i in [392c, 392(c+1)).  Each core:
  1. builds its [3136 x 392] block of exp(-0.5*sqdist) for both kernels
     directly on-chip: the full exponent comes out of one PE matmul over
     augmented features (D=4 spatial / D=5 bilateral), then ScalarE Exp;
  2. column sums via ones-matmuls -> reciprocal -> merge the two kernels
     into one C block (3 DVE ops with a PE-broadcast row);
  3. iterates: 28 accumulating matvec matmuls + row ops + sigmoid, then a
     1.5KB AllGather redistributes s0 to all cores (j is tiled as
     j = p*28 + t so the gathered vector DMA-rearranges into the [112,28]
     lhsT layout with per-partition contiguous reads).
Only the last iteration skips the collective and writes sigmoid(-d).
"""

import sys

for _p in ("/root/.axon_site/_ro/trn_rl_repo", "/opt/trn_rl_repo"):
    if _p not in sys.path:
        sys.path.append(_p)

import numpy as np

import concourse.bass as bass  # noqa: F401  (registers AP types)
import concourse.tile as tile
from concourse import bacc, mybir, bass_utils

F32 = mybir.dt.float32
BF16 = mybir.dt.bfloat16
AF = mybir.ActivationFunctionType
ALU = mybir.AluOpType

H = W = 56
N = H * W            # 3136 pixels
NC = 8               # cores
SHARD = N // NC      # 392 columns per core
P = 112              # j partition-tile height (112*28 == 3136)
T = 28               # number of j tiles
ITERS = 10
TH_ALPHA, TH_BETA, TH_GAMMA = 160.0, 3.0, 3.0

_BANK = 512          # one PSUM bank, in f32 elements
_GRP = 3             # exponent tiles batched per ScalarE Exp call
_WARM = 72           # HAM warm-keeping matmuls issued under each collective
_CSC = 4096.0        # fp8 prescale on C (exact power of two)
_CPAD = 392          # fp8 cmat per-tile column pitch
_GH = T // 2         # DoubleRow groups per matvec (14)


def _build(a_val: float, b_val: float) -> "bacc.Bacc":
    nc = bacc.Bacc("TRN2", target_bir_lowering=False, debug=False,
                   num_devices=NC)

    F8 = mybir.dt.float8e4
    fa_in = nc.dram_tensor("fa", [128, 2 * N], BF16, kind="ExternalInput").ap()
    fb_in = nc.dram_tensor("fb", [128, 2 * SHARD], BF16,
                           kind="ExternalInput").ap()
    u_in = nc.dram_tensor("u", [2, SHARD], BF16, kind="ExternalInput").ap()
    one2_in = nc.dram_tensor("one2", [2, 1], BF16, kind="ExternalInput").ap()
    s0_in = nc.dram_tensor("s0", [P, 32], F8, kind="ExternalInput").ap()
    onec_in = nc.dram_tensor("onec", [P, 1], BF16, kind="ExternalInput").ap()
    oner_in = nc.dram_tensor("oner", [1, P], F32, kind="ExternalInput").ap()
    out = nc.dram_tensor("out", [1, SHARD], F32, kind="ExternalOutput").ap()
    sink = nc.dram_tensor("sink", [1, 1], F32, kind="ExternalOutput").ap()
    # collective buffers: Local input, Shared output (fast HBM-HBM path).
    # The throwaway warmup collective gets its own pair so iteration 0's
    # input DMA never waits on the entry-barrier-delayed warmup read.
    di_d = [nc.dram_tensor(f"di{k}", [SHARD], F8, kind="Internal").ap()
            for k in range(3)]
    do_d = [nc.dram_tensor(f"do{k}", [N], F8, kind="Internal",
                           addr_space="Shared").ap()
            for k in range(3)]

    groups = [list(range(g, min(g + _GRP, T))) for g in range(0, T, _GRP)]

    with tile.TileContext(nc) as tc:
        with (
            tc.tile_pool(name="const", bufs=1) as cpool,
            tc.tile_pool(name="emat", bufs=1) as epool,
            tc.tile_pool(name="row", bufs=2) as rpool,
            tc.tile_pool(name="sten", bufs=2) as spool,
            tc.tile_pool(name="dram", bufs=2, space="DRAM") as dpool,
        ):
            # exponent-feature operands are zero-padded to 128 contraction
            # rows: a 4/5-row matmul doesn't register as PE activity, so the
            # HAM clock gate keeps the whole construction at 1.2 GHz.
            # Host sends the pad rows pre-zeroed (DMA engines are idle at
            # startup, the DVE memsets were on the critical path), and the
            # four operands ride in two DMAs: [bilateral | spatial].
            fa_t = cpool.tile([128, 2 * N], BF16, tag="fa")
            nc.sync.dma_start(fa_t[:], fa_in[:])
            fb_t = cpool.tile([128, 2 * SHARD], BF16, tag="fb")
            nc.sync.dma_start(fb_t[:], fb_in[:])
            u_t = cpool.tile([2, SHARD], BF16, tag="u")
            nc.sync.dma_start(u_t[:], u_in[:])
            ones2 = cpool.tile([2, 1], BF16, tag="one2")
            nc.sync.dma_start(ones2[:], one2_in[:])
            s0_t = cpool.tile([P, 32], F8, tag="s0")
            nc.sync.dma_start(s0_t[:], s0_in[:])
            ones_col = cpool.tile([P, 1], BF16, tag="onec")
            nc.sync.dma_start(ones_col[:], onec_in[:])
            ones_row = cpool.tile([1, P], F32, tag="oner")
            nc.sync.dma_start(ones_row[:], oner_in[:])

            # throwaway AllGather: absorbs the ~40us collectives entry
            # barrier + ~18us first-op warmup under the construction phase
            nc.sync.dma_start(
                di_d[2][:], s0_in[:].rearrange("p t -> (p t)")[0:SHARD])
            nc.gpsimd.collective_compute(
                "AllGather", ALU.bypass,
                replica_groups=[list(range(NC))],
                ins=[di_d[2][:].opt()], outs=[do_d[2][:].opt()],
            )

            ebl = epool.tile([P, T * SHARD], BF16, tag="ebl")
            cmat = epool.tile([P, T * _CPAD], F8, tag="cmat")

            # ---- phase 1: exponent matmuls + exp + column sums ----
            # bilateral pass first: its serial tail (colsum -> recip ->
            # rb broadcast -> ebl*rb) overlaps the spatial pass, whose exp
            # (with ln(CSC*|A|/nsp) absorbed as two feature rows) lands
            # directly in the fp8 cmat.
            with (
                tc.tile_pool(name="psg", bufs=2, space="PSUM") as psg,
                tc.tile_pool(name="pss", bufs=1, space="PSUM") as pss,
            ):
                cs_bl = pss.tile([1, SHARD], F32, tag="cs_bl")

                def exp_pass(aoff, boff, dst3, cs, scale):
                    for grp in groups:
                        pg = psg.tile([P, _GRP * _BANK], F32, tag="grp")
                        for k, t in enumerate(grp):
                            nc.tensor.matmul(
                                pg[:, k * _BANK : k * _BANK + SHARD],
                                fa_t[:, aoff + t * P : aoff + (t + 1) * P],
                                fb_t[:, boff : boff + SHARD],
                                start=True, stop=True,
                                skip_group_check=True,
                            )
                        ln = len(grp)
                        src = pg[:].rearrange("p (k f) -> p k f", f=_BANK)[
                            :, 0:ln, 0:SHARD]
                        nc.scalar.activation(dst3[:, grp[0] : grp[0] + ln, :],
                                             src, AF.Exp, scale=scale)
                        for t in (grp if cs is not None else []):
                            nc.tensor.matmul(
                                cs[:],
                                ones_col[:],
                                ebl[:, t * SHARD : (t + 1) * SHARD],
                                start=(t == 0), stop=(t == T - 1),
                                skip_group_check=True,
                            )

                b3 = ebl[:].rearrange("p (k f) -> p k f", f=SHARD)
                c3 = cmat[:].rearrange("p (k f) -> p k f", f=_CPAD
                                       )[:, :, 0:SHARD]
                exp_pass(0, 0, b3, cs_bl, -1.0 / 6.0)

                rb_row = cpool.tile([1, SHARD], F32, tag="rb")
                nc.vector.reciprocal(rb_row[:], cs_bl[:])
                nc.scalar.mul(rb_row[:], rb_row[:], float(b_val * _CSC))

                rb_bc = pss.tile([P, SHARD], F32, tag="rbbc")
                nc.tensor.matmul(rb_bc[:], ones_row[:], rb_row[:],
                                 start=True, stop=True, skip_group_check=True)
                rb_sb = cpool.tile([P, SHARD], BF16, tag="rbsb")
                nc.vector.tensor_copy(rb_sb[:], rb_bc[:])

                exp_pass(N, SHARD, c3, None, 1.0 / 9.0)

                # merge C = Esp' +/- Ebl*RB, split in pair-halves so the
                # first matvec groups can start before the second half lands
                c4 = cmat[:].rearrange("p (e q f) -> p e q f", q=_GH,
                                       f=_CPAD)[:, :, :, 0:SHARD]
                b4 = ebl[:].rearrange("p (e q f) -> p e q f", q=_GH, f=SHARD)
                r4 = rb_sb[:].rearrange("p (a b f) -> p a b f", a=1, b=1)
                for h in range(2):
                    q0, q1 = (h * _GH) // 2, ((h + 1) * _GH) // 2
                    nq = q1 - q0
                    bh = b4[:, :, q0:q1, :]
                    ch = c4[:, :, q0:q1, :]
                    nc.vector.tensor_mul(
                        bh, bh, r4.broadcast_to([P, 2, nq, SHARD]))
                    if a_val >= 0.0:
                        nc.vector.tensor_add(ch, ch, bh)
                    else:
                        nc.vector.tensor_sub(ch, bh, ch)

            # ---- phase 3: CRF mean-field iterations ----
            # psum accumulates -CSC*d = (-CSC*u seed) + sum_t CSC*C^T s.
            # fp8 DoubleRow matvec: group g contracts j-tiles (g, g+_GH);
            # s is stored [P, 2, 16] (14 used + 2 pad, 16B subtile step).
            with (
                tc.tile_pool(name="psv", bufs=2, space="PSUM") as psv,
                tc.tile_pool(name="psd", bufs=1, space="PSUM") as psd,
            ):
                dummy = psd.tile([1, SHARD], F32, tag="dummy")
                cm3 = cmat[:].rearrange("p (e g) -> p e g", e=2,
                                        g=_GH * _CPAD)
                DR = mybir.MatmulPerfMode.DoubleRow

                def seed(vt):
                    nc.tensor.matmul(
                        vt[:], ones2[:], u_t[:],
                        start=True, stop=False, skip_group_check=True,
                    )

                s_cur = s0_t
                v = psv.tile([1, SHARD], F32, tag="v")
                seed(v)
                for it in range(ITERS):
                    s3 = s_cur[:].rearrange("p (e g) -> p e g", e=2, g=16)
                    for g in range(_GH):
                        nc.tensor.matmul(
                            v[:],
                            s3[:, :, g : g + 1],
                            cm3[:, :, g * _CPAD : g * _CPAD + SHARD],
                            start=False, stop=(g == _GH - 1),
                            perf_mode=DR,
                            skip_group_check=True,
                        )
                    if it < ITERS - 1:
                        s_row = rpool.tile([1, SHARD], F8, tag="srow")
                        nc.scalar.activation(s_row[:], v[:], AF.Sigmoid,
                                             scale=-1.0 / _CSC)
                        di, do = di_d[it % 2], do_d[it % 2]
                        # scalar-issued DMA: no cross-engine semaphore hop
                        # between the sigmoid and the collective input
                        nc.scalar.dma_start(
                            di[:].rearrange("(a b) -> a b", a=1), s_row[:])
                        nc.gpsimd.collective_compute(
                            "AllGather", ALU.bypass,
                            replica_groups=[list(range(NC))],
                            ins=[di[:].opt()], outs=[do[:].opt()],
                        )
                        # keep the PE HAM-warm through the collective gap.
                        # The first ("linker") matmul reads s_row, so the
                        # whole WAW-chained dummy block is ordered after the
                        # sigmoid — it cannot interleave into the matvec
                        # accumulation and delay v's ready semaphore.  The
                        # next iteration's u-seed hides under the collective
                        # too, right behind the linker.
                        nc.tensor.matmul(
                            dummy[:], s_row[0:1, 0:1], s_row[:],
                            start=True, stop=True, skip_group_check=True,
                        )
                        v = psv.tile([1, SHARD], F32, tag="v")
                        seed(v)
                        for w in range(_WARM):
                            c0 = (w % T) * _CPAD
                            nc.tensor.matmul(
                                dummy[:],
                                s_cur[:, (w % 28) : (w % 28) + 1],
                                cmat[:, c0 : c0 + SHARD],
                                start=True, stop=True,
                                skip_group_check=True,
                            )
                        s_nxt = spool.tile([P, 32], F8, tag="s")
                        nc.sync.dma_start(
                            s_nxt[:].rearrange("p (e g) -> p e g",
                                               e=2, g=16)[:, :, 0:_GH],
                            do[:].rearrange("(p e g) -> p e g", e=2, g=_GH))
                        s_cur = s_nxt
                    else:
                        o_row = rpool.tile([1, SHARD], F32, tag="orow")
                        nc.scalar.activation(o_row[:], v[:], AF.Sigmoid,
                                             scale=1.0 / _CSC)
                        nc.scalar.dma_start(out[:], o_row[:])
                sink_row = rpool.tile([1, 1], F32, tag="sink")
                nc.vector.tensor_copy(sink_row[:], dummy[0:1, 0:1])
                nc.sync.dma_start(sink[:], sink_row[:])

    nc.compile()
    return nc


def _host_prep(inputs, spatial_ker_weights, bilateral_ker_weights,
               compatibility_matrix):
    unary = np.asarray(inputs[0], dtype=np.float64)
    gray = np.asarray(inputs[1], dtype=np.float64)
    sp_w = np.asarray(spatial_ker_weights, dtype=np.float64)
    bl_w = np.asarray(bilateral_ker_weights, dtype=np.float64)
    compat = np.asarray(compatibility_matrix, dtype=np.float64)

    dsp = sp_w[:, 0] - sp_w[:, 1]
    dbl = bl_w[:, 0] - bl_w[:, 1]
    c0 = sp_w[:, 1] + bl_w[:, 1]
    dc = compat[0, :] - compat[1, :]
    a_val = float(dc @ dsp)
    b_val = float(dc @ dbl)
    g_val = float(dc @ c0)

    ys, xs = np.meshgrid(np.arange(H, dtype=np.float64),
                         np.arange(W, dtype=np.float64), indexing="ij")
    x = xs.ravel()
    y = ys.ravel()
    gf = gray.ravel() * 255.0

    import ml_dtypes
    _mld = ml_dtypes
    one = np.ones(N, dtype=np.float64)

    def bf(v):
        return np.asarray(v, dtype=_mld.bfloat16).astype(np.float64)

    def split3(v):
        a = bf(v)
        b = bf(v - a)
        c = bf(v - a - b)
        return a, b, c

    # spatial norm is a Kronecker product: nsp[(y,x)] = ry[y]*rx[x]
    idx = np.arange(H, dtype=np.float64)
    g1d = np.exp(-0.5 * ((idx[None, :] - idx[:, None]) / TH_GAMMA) ** 2)
    r1d = g1d.sum(axis=1)
    nsp = (r1d[y.astype(int)] * r1d[x.astype(int)])

    # spatial exponent in bf16-exact integer arithmetic, scaled by 1/9 at
    # the Exp activation; the fp8 prescale and the column norm are folded
    # in as a 9*ln(CSC*|A|/nsp) hi/lo feature pair:
    # presc = xj*xi + yj*yi - (x^2+y^2)/2 terms + ln rows
    ssp_i = 0.5 * (x * x + y * y)                 # multiples of 0.5
    sp_hi = bf(-ssp_i)
    sp_lo = -ssp_i - sp_hi                        # exact in bf16
    lr = 9.0 * np.log(np.maximum(4096.0 * abs(a_val) / nsp, 1e-280))
    lr = np.maximum(lr, -2000.0)
    lr_hi = bf(lr)
    lr_lo = lr - lr_hi

    asp_g = np.stack([x, y, one, one, sp_hi, sp_lo, one, one], axis=0)
    bsp_g = np.stack([x, y, sp_hi, sp_lo, one, one, lr_hi, lr_lo], axis=0)

    # bilateral exponent on bf16 features: presc = ssq_i + ssq_j
    # - 2*(xp_i xp_j + yp_i yp_j + g_i g_j), scaled by -1/6 at the Exp.
    # g and ssq are 3-way bf16 splits so every product is exact in f32;
    # validated max exponent error ~2e-3.
    s3f = np.sqrt(3.0)
    xp = bf(s3f * x / TH_ALPHA)
    yp = bf(s3f * y / TH_ALPHA)
    g1, g2, g3 = split3(gf)
    gs = g1 + g2 + g3
    ssq = xp * xp + yp * yp + gs * gs
    s1, s2, sr = split3(ssq)

    abl_g = np.stack([xp, yp, g1, g1, g2, g1, g3, g2,
                      s1, s2, sr, one, one, one], axis=0)
    bbl_g = np.stack([-2 * xp, -2 * yp, -2 * g1, -2 * g2, -2 * g1,
                      -2 * g3, -2 * g1, -2 * g2,
                      one, one, one, s1, s2, sr], axis=0)

    # device j-tiling: lhsT column t*P + p  <->  global j = p*T + t
    j_order = np.arange(N).reshape(P, T).T.ravel()

    def pad128(m, cols):
        out = np.zeros((128, cols), dtype=_mld.bfloat16)
        out[: m.shape[0]] = m.astype(_mld.bfloat16)
        return out

    # fa/fb pack [bilateral | spatial] so each rides in a single DMA
    fa_d = np.concatenate(
        [pad128(abl_g[:, j_order], N), pad128(asp_g[:, j_order], N)], axis=1)

    def hilo(v):
        hi = np.asarray(v, dtype=_mld.bfloat16).astype(np.float64)
        lo = np.asarray(v - hi, dtype=_mld.bfloat16)
        return np.stack([hi.astype(_mld.bfloat16), lo], axis=0)

    u_flat = unary.ravel()
    u_full = hilo(-4096.0 * ((1.0 - 2.0 * u_flat) - g_val))    # [2, N] bf16
    s0_pt = (1.0 - u_flat).reshape(P, T)
    s0_dev = np.zeros((P, 32), dtype=ml_dtypes.float8_e4m3fn)
    s0_dev[:, 0:14] = s0_pt[:, 0:14].astype(ml_dtypes.float8_e4m3fn)
    s0_dev[:, 16:30] = s0_pt[:, 14:28].astype(ml_dtypes.float8_e4m3fn)

    in_maps = []
    for c in range(NC):
        sl = slice(c * SHARD, (c + 1) * SHARD)
        in_maps.append({
            "fa": fa_d,
            "fb": np.concatenate([pad128(bbl_g[:, sl], SHARD),
                                  pad128(bsp_g[:, sl], SHARD)], axis=1),
            "u": np.ascontiguousarray(u_full[:, sl]),
            "s0": s0_dev,
            "onec": np.ones((P, 1), dtype=ml_dtypes.bfloat16),
            "one2": np.ones((2, 1), dtype=ml_dtypes.bfloat16),
            "oner": np.ones((1, P), dtype=np.float32),
        })
    return a_val, b_val, in_maps


_CACHE = {}


def kernel(inputs, spatial_ker_weights, bilateral_ker_weights,
           compatibility_matrix, _want_results=False):
    a_val, b_val, in_maps = _host_prep(
        inputs, spatial_ker_weights, bilateral_ker_weights,
        compatibility_matrix)

    key = (a_val, b_val)
    if key not in _CACHE:
        _CACHE[key] = _build(a_val, b_val)
    nc = _CACHE[key]

    res = bass_utils.run_bass_kernel_spmd(nc, in_maps, list(range(NC)))
    prob = np.concatenate([res.results[c]["out"][0] for c in range(NC)])
    out = prob.reshape(1, H, W).astype(np.float32)
    if _want_results:
        return out, nc, in_maps
    return out


if __name__ == "__main__":
    rng = np.random.default_rng(0)
    demo = {
        "inputs": rng.random((2, H, W)).astype(np.float32),
        "spatial_ker_weights":
            (rng.random((2, 2)).astype(np.float32) - 0.5) * 0.1,
        "bilateral_ker_weights":
            (rng.random((2, 2)).astype(np.float32) - 0.5) * 0.1,
        "compatibility_matrix":
            (rng.random((2, 2)).astype(np.float32) - 0.5) * 0.1,
    }
    print(kernel(**demo).shape)

